# revision 5
# baseline (speedup 1.0000x reference)
"""Causal GQA self-attention (B=2, T=2048, C=2048, H=16, KVH=4, HD=128) on 8 TRN2
NeuronCores.

Sharding: one (batch, kv-head-group) pair per core — 2 batches x 4 kv groups = 8
cores. Each core computes, for its batch b and kv group g:
  q = x[b] @ wq[:, g*512:(g+1)*512]      (4 query heads)
  k = x[b] @ wk[:, g*128:(g+1)*128]
  v = x[b] @ wv[:, g*128:(g+1)*128]
  q,k -> RoPE -> RMS-norm; causal softmax(q k^T / sqrt(hd)) @ v
  y_partial = attn_out @ wo[g*512:(g+1)*512, :]
Host sums the 4 partial y's per batch (the O-projection contraction).

All matmuls run as fp32r (full PE rate at moving-dim >= 256; verified bit-identical
to the fp32 path on this hardware) except the attention-probability side
(P/V/wo/aoT) which runs bf16 — safe post-softmax.
"""
import numpy as np
import ml_dtypes

import concourse.bass as bass
import concourse.mybir as mybir
import concourse.tile as tile
from concourse import bacc
from concourse.bass_utils import run_bass_kernel_spmd

P = 128          # partitions / head dim
T = 2048         # sequence length
C = 2048         # model dim
NH = 4           # query heads per core (n_rep)
NT = T // P      # 16 t-chunks
NCC = C // P     # 16 contraction chunks
NT4 = 4          # t-chunks of 512
HD = 128
EPS = 1e-5
SCALE = 1.0 / np.sqrt(np.float32(HD))

f32 = mybir.dt.float32
f32r = mybir.dt.float32r
bf16 = mybir.dt.bfloat16
Exp = mybir.ActivationFunctionType.Exp
Sqrt = mybir.ActivationFunctionType.Sqrt
Square = mybir.ActivationFunctionType.Square
MULT = mybir.AluOpType.mult
ADD = mybir.AluOpType.add


def bcast_mid(ap, n):
    """(P, F) AP -> (P, n, F) with broadcast middle dim."""
    return bass.AP(tensor=ap.tensor, offset=ap.offset,
                   ap=[list(ap.ap[0]), [0, n], list(ap.ap[1])])


def _phase_a(nc, tc, io, sb):
    """QKV projection + RoPE + RMS + transposes -> qT_all, kT_all, v_all."""
    with tc.tile_pool(name="csp", bufs=2) as csp, \
         tc.tile_pool(name="xp", bufs=4) as xp, \
         tc.tile_pool(name="qw", bufs=2) as qw, \
         tc.tile_pool(name="rp", bufs=4) as rp, \
         tc.tile_pool(name="st", bufs=8) as st, \
         tc.tile_pool(name="psA", bufs=2, space="PSUM") as psA, \
         tc.tile_pool(name="psAT", bufs=2, space="PSUM") as psAT:
        for t in range(NT):
            ts = slice(t * P, (t + 1) * P)
            psum_q = psA.tile([P, NH * HD], f32, tag="q", name="psum_q")
            psum_kv = psA.tile([P, 2 * HD], f32, tag="kv", name="psum_kv")
            for c in range(NCC):
                xt = xp.tile([P, P], f32r, tag="xt", name="xt")
                nc.sync.dma_start(out=xt, in_=io["xT"][c * P:(c + 1) * P, ts])
                nc.tensor.matmul(psum_q, xt, sb["wq_s"][:, c, :],
                                 start=(c == 0), stop=(c == NCC - 1))
                nc.tensor.matmul(psum_kv, xt, sb["wkv_s"][:, c, :],
                                 start=(c == 0), stop=(c == NCC - 1))
            # evacuate
            q_sb = qw.tile([P, NH, HD], f32, tag="q_sb", name="q_sb")
            nc.scalar.copy(q_sb, psum_q.rearrange("p (h d) -> p h d", h=NH))
            k_sb = qw.tile([P, HD], f32, tag="k_sb", name="k_sb")
            nc.scalar.copy(k_sb, psum_kv[:, 0:HD])
            nc.scalar.copy(sb["v_all"][:, t, :], psum_kv[:, HD:2 * HD])

            # RMS statistics from pre-rope values (rope is a rotation: it
            # preserves per-row L2 norms, so mean(q^2) is unchanged by it)
            msq = st.tile([P, NH + 1], f32, tag="msq", name="msq")
            scr = st.tile([P, HD], f32, tag="scr", name="scr")
            for h in range(NH):
                nc.scalar.activation(scr, psum_q[:, h * HD:(h + 1) * HD], Square,
                                     accum_out=msq[:, h:h + 1])
            nc.scalar.activation(scr, psum_kv[:, 0:HD], Square,
                                 accum_out=msq[:, NH:NH + 1])
            # r = rsqrt(ms + eps), Newton-refined:
            #   u = 1/(ms+eps) [DVE, accurate]; r0 = sqrt(u) [ACT, coarse]
            #   r = r0 * (1.5 - 0.5 * (ms+eps) * r0^2)
            xme = st.tile([P, NH + 1], f32, tag="xme", name="xme")
            nc.vector.tensor_scalar(xme, msq, 1.0 / HD, float(EPS), MULT, ADD)
            u = st.tile([P, NH + 1], f32, tag="u", name="u")
            nc.vector.reciprocal(u, xme)
            r0 = st.tile([P, NH + 1], f32, tag="r0", name="r0")
            nc.scalar.activation(r0, u, Sqrt)
            t0 = st.tile([P, NH + 1], f32, tag="t0", name="t0")
            nc.vector.tensor_mul(t0, r0, r0)
            nc.vector.tensor_mul(t0, t0, xme)
            nc.vector.tensor_scalar(t0, t0, -0.5, 1.5, MULT, ADD)
            rr = st.tile([P, NH + 1], f32, tag="rr", name="rr")
            nc.vector.tensor_mul(rr, r0, t0)

            # RoPE (halves-split): out1 = x1*cos + x2*sin ; out2 = x2*cos - x1*sin
            cos_t = csp.tile([P, HD // 2], f32, tag="cos", name="cos_t")
            nc.sync.dma_start(out=cos_t, in_=io["cosx"][ts, :])
            sin_t = csp.tile([P, HD // 2], f32, tag="sin", name="sin_t")
            nc.sync.dma_start(out=sin_t, in_=io["sinx"][ts, :])
            cos_q = bcast_mid(cos_t, NH)
            sin_q = bcast_mid(sin_t, NH)

            qr = qw.tile([P, NH, HD], f32, tag="qr", name="qr")
            ta = rp.tile([P, NH, HD // 2], f32, tag="ta", name="ta")
            tb = rp.tile([P, NH, HD // 2], f32, tag="tb", name="tb")
            q1, q2 = q_sb[:, :, 0:HD // 2], q_sb[:, :, HD // 2:HD]
            nc.vector.tensor_mul(ta, q1, cos_q)
            nc.vector.tensor_mul(tb, q2, sin_q)
            nc.vector.tensor_add(qr[:, :, 0:HD // 2], ta, tb)
            nc.vector.tensor_mul(ta, q2, cos_q)
            nc.vector.tensor_mul(tb, q1, sin_q)
            nc.vector.tensor_sub(qr[:, :, HD // 2:HD], ta, tb)

            kr = qw.tile([P, HD], f32, tag="kr", name="kr")
            ka = rp.tile([P, HD // 2], f32, tag="ka", name="ka")
            kb = rp.tile([P, HD // 2], f32, tag="kb", name="kb")
            k1, k2 = k_sb[:, 0:HD // 2], k_sb[:, HD // 2:HD]
            nc.vector.tensor_mul(ka, k1, cos_t)
            nc.vector.tensor_mul(kb, k2, sin_t)
            nc.vector.tensor_add(kr[:, 0:HD // 2], ka, kb)
            nc.vector.tensor_mul(ka, k2, cos_t)
            nc.vector.tensor_mul(kb, k1, sin_t)
            nc.vector.tensor_sub(kr[:, HD // 2:HD], ka, kb)

            # apply RMS scale (q also gets the 1/sqrt(hd) attention scale)
            qhat = qw.tile([P, NH, HD], f32r, tag="qhat", name="qhat")
            for h in range(NH):
                nc.vector.tensor_scalar(qhat[:, h, :], qr[:, h, :],
                                        rr[:, h:h + 1], float(SCALE), MULT, MULT)
            khat = qw.tile([P, HD], f32r, tag="khat", name="khat")
            nc.vector.tensor_scalar(khat, kr, rr[:, NH:NH + 1], None, MULT)

            # transpose to (d, t) layouts for the attention matmuls
            for h in range(NH):
                ps_t = psAT.tile([P, P], f32r, tag="t", name="ps_t")
                nc.tensor.matmul(ps_t, qhat[:, h, :], sb["ident_s"],
                                 is_transpose=True, start=True, stop=True)
                nc.scalar.copy(sb["qT_all"][:, h, ts], ps_t)
            ps_t = psAT.tile([P, P], f32r, tag="t", name="ps_t2")
            nc.tensor.matmul(ps_t, khat, sb["ident_s"],
                             is_transpose=True, start=True, stop=True)
            nc.scalar.copy(sb["kT_all"][:, ts], ps_t)


def _phase_b(nc, tc, io, sb):
    """Attention: softmax(q k^T) v -> aoT_s, per (head, 512-wide t block)."""
    with tc.tile_pool(name="pp", bufs=6) as pp, \
         tc.tile_pool(name="ptp", bufs=3) as ptp, \
         tc.tile_pool(name="sst", bufs=8) as sst, \
         tc.tile_pool(name="psS", bufs=3, space="PSUM") as psS, \
         tc.tile_pool(name="psT", bufs=2, space="PSUM") as psT, \
         tc.tile_pool(name="psO", bufs=2, space="PSUM") as psO:
        for h in range(NH):
            for t4 in range(NT4):
                p_tiles = []
                for m in range(4):
                    ti = t4 * 4 + m
                    s_len = (ti + 1) * P
                    nch = (s_len + 511) // 512
                    p_ti = pp.tile([P, T], bf16, tag="p", name="p_ti")[:, :s_len]
                    rsp = sst.tile([P, 4], f32, tag="rsp", name="rsp")
                    for cj in range(nch):
                        w = min(512, s_len - cj * 512)
                        ps_s = psS.tile([P, 512], f32, tag="s",
                                        name="ps_s")[:, :w]
                        nc.tensor.matmul(
                            ps_s, sb["qT_all"][:, h, ti * P:(ti + 1) * P],
                            sb["kT_all"][:, cj * 512:cj * 512 + w],
                            start=True, stop=True)
                        if cj == nch - 1:  # causal mask on the diagonal block
                            nc.vector.tensor_add(
                                ps_s[:, w - P:w], ps_s[:, w - P:w],
                                sb["tril_s"])
                        nc.scalar.activation(
                            p_ti[:, cj * 512:cj * 512 + w], ps_s, Exp,
                            accum_out=rsp[:, cj:cj + 1])
                    rs = rsp[:, 0:1]
                    if nch > 1:
                        rs = sst.tile([P, 1], f32, tag="rs", name="rs")
                        nc.vector.tensor_add(rs, rsp[:, 0:1], rsp[:, 1:2])
                        for cj in range(2, nch):
                            nc.vector.tensor_add(rs, rs, rsp[:, cj:cj + 1])
                    rcp = sst.tile([P, 1], f32, tag="rcp", name="rcp")
                    nc.vector.reciprocal(rcp, rs)
                    nc.vector.tensor_scalar_mul(p_ti, p_ti, rcp)
                    p_tiles.append(p_ti)

                # PV: O^T[d, t512] accumulated over 128-wide s chunks
                ps_o = psO.tile([P, 512], f32, tag="o", name="ps_o")
                n_sj = t4 * 4 + 4
                for sj in range(n_sj):
                    pt = ptp.tile([P, 512], bf16, tag="pt", name="pt")
                    for m in range(4):
                        ti = t4 * 4 + m
                        if sj <= ti:
                            ps_t = psT.tile([P, P], bf16, tag="t", name="ps_tb")
                            nc.tensor.matmul(
                                ps_t, p_tiles[m][:, sj * P:(sj + 1) * P],
                                sb["identb_s"], is_transpose=True,
                                start=True, stop=True)
                            nc.scalar.copy(pt[:, m * P:(m + 1) * P], ps_t)
                        else:
                            nc.vector.memset(pt[:, m * P:(m + 1) * P], 0.0)
                    nc.tensor.matmul(ps_o, sb["v_all"][:, sj, :], pt,
                                     start=(sj == 0), stop=(sj == n_sj - 1))
                nc.scalar.copy(sb["aoT_s"][:, h, t4 * 512:(t4 + 1) * 512], ps_o)


def _phase_c(nc, tc, io, sb):
    """O-projection partial: y = aoT^T @ wo_g."""
    with tc.tile_pool(name="yp", bufs=3) as yp, \
         tc.tile_pool(name="psC", bufs=4, space="PSUM") as psC:
        for m in range(NT):
            for cc in range(4):
                ps_y = psC.tile([P, 512], f32, tag="y", name="ps_y")
                for h in range(NH):
                    nc.tensor.matmul(
                        ps_y, sb["aoT_s"][:, h, m * P:(m + 1) * P],
                        sb["wo_s"][:, h, cc * 512:(cc + 1) * 512],
                        start=(h == 0), stop=(h == NH - 1))
                y_sb = yp.tile([P, 512], f32, tag="y_sb", name="y_sb")
                nc.vector.tensor_copy(y_sb, ps_y)
                nc.sync.dma_start(
                    out=io["y"][m * P:(m + 1) * P, cc * 512:(cc + 1) * 512],
                    in_=y_sb)


def build_program(phases="ABC"):
    nc = bacc.Bacc("TRN2", target_bir_lowering=False, debug=False)

    io = {
        "xT": nc.dram_tensor("xT", [C, T], f32r, kind="ExternalInput").ap(),
        "wq": nc.dram_tensor("wq", [C, NH * HD], f32r, kind="ExternalInput").ap(),
        "wkv": nc.dram_tensor("wkv", [C, 2 * HD], f32r, kind="ExternalInput").ap(),
        "wo": nc.dram_tensor("wo", [NH * HD, C], bf16, kind="ExternalInput").ap(),
        "cosx": nc.dram_tensor("cosx", [T, HD // 2], f32, kind="ExternalInput").ap(),
        "sinx": nc.dram_tensor("sinx", [T, HD // 2], f32, kind="ExternalInput").ap(),
        "tril": nc.dram_tensor("tril", [P, P], f32, kind="ExternalInput").ap(),
        "ident": nc.dram_tensor("ident", [P, P], f32r, kind="ExternalInput").ap(),
        "identb": nc.dram_tensor("identb", [P, P], bf16, kind="ExternalInput").ap(),
        "y": nc.dram_tensor("y", [T, C], f32, kind="ExternalOutput").ap(),
    }

    with tile.TileContext(nc) as tc:
        with tc.tile_pool(name="const", bufs=1) as const:
            sb = {}
            sb["wq_s"] = const.tile([P, NCC, NH * HD], f32r, name="wq_s")
            nc.sync.dma_start(out=sb["wq_s"],
                              in_=io["wq"].rearrange("(c p) n -> p c n", p=P))
            sb["wkv_s"] = const.tile([P, NCC, 2 * HD], f32r, name="wkv_s")
            nc.sync.dma_start(out=sb["wkv_s"],
                              in_=io["wkv"].rearrange("(c p) n -> p c n", p=P))
            sb["wo_s"] = const.tile([P, NH, C], bf16, name="wo_s")
            nc.sync.dma_start(out=sb["wo_s"],
                              in_=io["wo"].rearrange("(h p) n -> p h n", p=P))
            sb["tril_s"] = const.tile([P, P], f32, name="tril_s")
            nc.sync.dma_start(out=sb["tril_s"], in_=io["tril"])
            sb["ident_s"] = const.tile([P, P], f32r, name="ident_s")
            nc.sync.dma_start(out=sb["ident_s"], in_=io["ident"])
            sb["identb_s"] = const.tile([P, P], bf16, name="identb_s")
            nc.sync.dma_start(out=sb["identb_s"], in_=io["identb"])

            sb["qT_all"] = const.tile([P, NH, T], f32r, name="qT_all")
            sb["kT_all"] = const.tile([P, T], f32r, name="kT_all")
            sb["v_all"] = const.tile([P, NT, HD], bf16, name="v_all")
            sb["aoT_s"] = const.tile([P, NH, T], bf16, name="aoT_s")

            if "A" in phases:
                _phase_a(nc, tc, io, sb)
            if "B" in phases:
                _phase_b(nc, tc, io, sb)
            if "C" in phases:
                _phase_c(nc, tc, io, sb)
            else:
                # debug output so the program still writes y
                with tc.tile_pool(name="dbg", bufs=1) as dbg:
                    d = dbg.tile([P, T], f32, name="d")
                    nc.vector.tensor_copy(d, sb["kT_all"].bitcast(f32))
                    nc.sync.dma_start(out=io["y"][0:P, :], in_=d)

    nc.compile()
    return nc


_PROG = None


def _get_prog():
    global _PROG
    if _PROG is None:
        _PROG = build_program()
    return _PROG


def make_in_maps(x, cos, sin, wq, wk, wv, wo):
    """Shard full inputs into 8 per-core input dicts."""
    cosf = np.ascontiguousarray(cos.reshape(T, HD // 2)).astype(np.float32)
    sinf = np.ascontiguousarray(sin.reshape(T, HD // 2)).astype(np.float32)
    ii, jj = np.indices((P, P))
    tril = np.where(jj <= ii, 0.0, -1e30).astype(np.float32)
    ident = np.eye(P, dtype=np.float32)
    identb = np.eye(P, dtype=np.float32).astype(ml_dtypes.bfloat16)

    in_maps = []
    for g in range(8):
        b, kv = divmod(g, 4)
        sl4 = slice(kv * NH * HD, (kv + 1) * NH * HD)   # 512 wide
        sl1 = slice(kv * HD, (kv + 1) * HD)             # 128 wide
        in_maps.append({
            "xT": np.ascontiguousarray(x[b].T).astype(np.float32),
            "wq": np.ascontiguousarray(wq[:, sl4]).astype(np.float32),
            "wkv": np.ascontiguousarray(
                np.concatenate([wk[:, sl1], wv[:, sl1]], axis=1)).astype(np.float32),
            "wo": np.ascontiguousarray(wo[sl4, :]).astype(ml_dtypes.bfloat16),
            "cosx": cosf, "sinx": sinf,
            "tril": tril, "ident": ident, "identb": identb,
        })
    return in_maps


def kernel(x, cos, sin, wq, wk, wv, wo, window_size=0):
    x = np.asarray(x); cos = np.asarray(cos); sin = np.asarray(sin)
    wq = np.asarray(wq); wk = np.asarray(wk); wv = np.asarray(wv)
    wo = np.asarray(wo)
    prog = _get_prog()
    in_maps = make_in_maps(x, cos, sin, wq, wk, wv, wo)
    res = run_bass_kernel_spmd(prog, in_maps, core_ids=list(range(8)))
    outs = [r["y"] for r in res.results]
    yfull = np.empty((2, T, C), dtype=np.float32)
    for b in range(2):
        yfull[b] = outs[4 * b] + outs[4 * b + 1] + outs[4 * b + 2] + outs[4 * b + 3]
    return yfull


# revision 25
# speedup vs baseline: 398.9360x; 398.9360x over previous
"""Causal GQA self-attention (B=2, T=2048, C=2048, H=16, KVH=4, HD=128) on 8 TRN2
NeuronCores.

Sharding: one (batch, kv-head-group) pair per core — 2 batches x 4 kv groups = 8
cores. Each core computes, for its batch b and kv group g:
  q = x[b] @ wq[:, g*512:(g+1)*512]      (4 query heads)
  k = x[b] @ wk[:, g*128:(g+1)*128]
  v = x[b] @ wv[:, g*128:(g+1)*128]
  q,k -> RoPE -> RMS-norm; causal softmax(q k^T / sqrt(hd)) @ v
  y_partial = attn_out @ wo[g*512:(g+1)*512, :]
Host sums the 4 partial y's per batch (the O-projection contraction).

All matmuls run as fp32r (full PE rate at moving-dim >= 256; verified bit-identical
to the fp32 path on this hardware) except the attention-probability side
(P/V/wo/aoT) which runs bf16 — safe post-softmax.
"""
import numpy as np
import ml_dtypes

import concourse.bass as bass
import concourse.mybir as mybir
import concourse.tile as tile
from concourse import bacc
from concourse.bass_utils import run_bass_kernel_spmd

P = 128          # partitions / head dim
T = 2048         # sequence length
C = 2048         # model dim
NH = 4           # query heads per core (n_rep)
NT = T // P      # 16 t-chunks
NCC = C // P     # 16 contraction chunks
NT4 = 4          # t-chunks of 512
HD = 128
EPS = 1e-5
SCALE = 1.0 / np.sqrt(np.float32(HD))

f32 = mybir.dt.float32
f32r = mybir.dt.float32r
bf16 = mybir.dt.bfloat16
Exp = mybir.ActivationFunctionType.Exp
Sqrt = mybir.ActivationFunctionType.Sqrt
Square = mybir.ActivationFunctionType.Square
MULT = mybir.AluOpType.mult
ADD = mybir.AluOpType.add


def bcast_mid(ap, n):
    """(P, F) AP -> (P, n, F) with broadcast middle dim."""
    return bass.AP(tensor=ap.tensor, offset=ap.offset,
                   ap=[list(ap.ap[0]), [0, n], list(ap.ap[1])])


def _phase_a(nc, tc, io, sb):
    """QKV projection + RoPE + RMS + transposes -> qT_all, kT_all, v_all."""
    with tc.tile_pool(name="csp", bufs=2) as csp, \
         tc.tile_pool(name="xp", bufs=4) as xp, \
         tc.tile_pool(name="qw", bufs=2) as qw, \
         tc.tile_pool(name="rp", bufs=4) as rp, \
         tc.tile_pool(name="st", bufs=8) as st, \
         tc.tile_pool(name="psA", bufs=1, space="PSUM") as psA:
        for tg in range(NT // 4):   # groups of 4 t-chunks: 2KB DMA lines
            q4 = psA.tile([P, 4, NH * HD], f32, tag="q4", name="q4")
            kv4 = psA.tile([P, 4, 512], f32, tag="kv4", name="kv4")
            for c in range(NCC):
                xt4 = xp.tile([P, 512], f32r, tag="xt", name="xt4")
                nc.sync.dma_start(
                    out=xt4,
                    in_=io["xT"][c * P:(c + 1) * P, tg * 512:(tg + 1) * 512])
                for u in range(4):
                    nc.tensor.matmul(q4[:, u, :], xt4[:, u * P:(u + 1) * P],
                                     sb["wq_s"][:, c, :],
                                     start=(c == 0), stop=(c == NCC - 1))
                    nc.tensor.matmul(kv4[:, u, 0:2 * HD],
                                     xt4[:, u * P:(u + 1) * P],
                                     sb["wkv_s"][:, c, :],
                                     start=(c == 0), stop=(c == NCC - 1))
          # per t-chunk post-processing
          # (kept at same indent level via inner loop)
            qhats, khats = [], []
            for u in range(4):
                t = tg * 4 + u
                ts = slice(t * P, (t + 1) * P)
                psum_q = q4[:, u, :]
                psum_kv = kv4[:, u, 0:2 * HD]
                # evacuate
                q_sb = qw.tile([P, NH, HD], f32, tag="q_sb", name="q_sb")
                nc.scalar.copy(q_sb.rearrange("p h d -> p (h d)"), psum_q)
            k_sb = qw.tile([P, HD], f32, tag="k_sb", name="k_sb")
            nc.scalar.copy(k_sb, psum_kv[:, 0:HD])
            nc.scalar.copy(sb["v_all"][:, t, :], psum_kv[:, HD:2 * HD])

            # RMS statistics from pre-rope values (rope is a rotation: it
            # preserves per-row L2 norms, so mean(q^2) is unchanged by it)
            msq = st.tile([P, NH + 1], f32, tag="msq", name="msq")
            scr = st.tile([P, HD], f32, tag="scr", name="scr")
            for h in range(NH):
                nc.vector.scalar_tensor_tensor(
                    out=scr, in0=q_sb[:, h, :], scalar=1.0, in1=q_sb[:, h, :],
                    op0=MULT, op1=MULT, accum_out=msq[:, h:h + 1])
            nc.vector.scalar_tensor_tensor(
                out=scr, in0=k_sb, scalar=1.0, in1=k_sb,
                op0=MULT, op1=MULT, accum_out=msq[:, NH:NH + 1])
            # r = rsqrt(ms + eps), Newton-refined:
            #   u = 1/(ms+eps) [DVE, accurate]; r0 = sqrt(u) [ACT, coarse]
            #   r = r0 * (1.5 - 0.5 * (ms+eps) * r0^2)
            xme = st.tile([P, NH + 1], f32, tag="xme", name="xme")
            nc.vector.tensor_scalar(xme, msq, 1.0 / HD, float(EPS), MULT, ADD)
            u = st.tile([P, NH + 1], f32, tag="u", name="u")
            nc.vector.reciprocal(u, xme)
            r0 = st.tile([P, NH + 1], f32, tag="r0", name="r0")
            nc.scalar.activation(r0, u, Sqrt)
            t0 = st.tile([P, NH + 1], f32, tag="t0", name="t0")
            nc.vector.tensor_mul(t0, r0, r0)
            nc.vector.tensor_mul(t0, t0, xme)
            nc.vector.tensor_scalar(t0, t0, -0.5, 1.5, MULT, ADD)
            rr = st.tile([P, NH + 1], f32, tag="rr", name="rr")
            nc.vector.tensor_mul(rr, r0, t0)

            # RoPE (halves-split): out1 = x1*cos + x2*sin ; out2 = x2*cos - x1*sin
            cos_t = csp.tile([P, HD // 2], f32, tag="cos", name="cos_t")
            nc.sync.dma_start(out=cos_t, in_=io["cosx"][ts, :])
            sin_t = csp.tile([P, HD // 2], f32, tag="sin", name="sin_t")
            nc.sync.dma_start(out=sin_t, in_=io["sinx"][ts, :])
            cos_q = bcast_mid(cos_t, NH)
            sin_q = bcast_mid(sin_t, NH)

            qr = qw.tile([P, NH, HD], f32, tag="qr", name="qr")
            ta = rp.tile([P, NH, HD // 2], f32, tag="ta", name="ta")
            tb = rp.tile([P, NH, HD // 2], f32, tag="tb", name="tb")
            q1, q2 = q_sb[:, :, 0:HD // 2], q_sb[:, :, HD // 2:HD]
            nc.vector.tensor_mul(ta, q1, cos_q)
            nc.vector.tensor_mul(tb, q2, sin_q)
            nc.vector.tensor_add(qr[:, :, 0:HD // 2], ta, tb)
            nc.vector.tensor_mul(ta, q2, cos_q)
            nc.vector.tensor_mul(tb, q1, sin_q)
            nc.vector.tensor_sub(qr[:, :, HD // 2:HD], ta, tb)

            kr = qw.tile([P, HD], f32, tag="kr", name="kr")
            ka = rp.tile([P, HD // 2], f32, tag="ka", name="ka")
            kb = rp.tile([P, HD // 2], f32, tag="kb", name="kb")
            k1, k2 = k_sb[:, 0:HD // 2], k_sb[:, HD // 2:HD]
            nc.vector.tensor_mul(ka, k1, cos_t)
            nc.vector.tensor_mul(kb, k2, sin_t)
            nc.vector.tensor_add(kr[:, 0:HD // 2], ka, kb)
            nc.vector.tensor_mul(ka, k2, cos_t)
            nc.vector.tensor_mul(kb, k1, sin_t)
            nc.vector.tensor_sub(kr[:, HD // 2:HD], ka, kb)

            # apply RMS scale (q also gets the 1/sqrt(hd) attention scale)
            qhat = qw.tile([P, NH, HD], f32r, tag="qhat", name="qhat", bufs=5)
            for h in range(NH):
                nc.vector.tensor_scalar(qhat[:, h, :], qr[:, h, :],
                                        rr[:, h:h + 1], float(SCALE), MULT, MULT)
            khat = qw.tile([P, HD], f32r, tag="khat", name="khat", bufs=5)
            nc.vector.tensor_scalar(khat, kr, rr[:, NH:NH + 1], None, MULT)

            # transpose to (d, t) layouts for the attention matmuls;
            # 4 q-head transposes share one PSUM tile -> single strided evac
            ps_t4 = psAT.tile([P, NH, P], f32r, tag="t", name="ps_t4")
            for h in range(NH):
                nc.tensor.matmul(ps_t4[:, h, :], qhat[:, h, :], sb["ident_s"],
                                 is_transpose=True, start=True, stop=True)
            nc.scalar.copy(sb["qT_all"][:, :, ts], ps_t4)
            ps_t = psAT.tile([P, P], f32r, tag="t2", name="ps_t2")
            nc.tensor.matmul(ps_t, khat, sb["ident_s"],
                             is_transpose=True, start=True, stop=True)
            nc.scalar.copy(sb["kT_all"][:, ts], ps_t)


def _phase_b(nc, tc, io, sb):
    """Attention + O-projection, t4-outer so PE work from the O-projection of
    block t4 overlaps the attention dependency chains of block t4+1."""
    with tc.tile_pool(name="pp", bufs=8) as pp, \
         tc.tile_pool(name="ptp", bufs=4) as ptp, \
         tc.tile_pool(name="sst", bufs=8) as sst, \
         tc.tile_pool(name="yp", bufs=3) as yp, \
         tc.tile_pool(name="psS", bufs=3, space="PSUM") as psS, \
         tc.tile_pool(name="psT", bufs=2, space="PSUM") as psT, \
         tc.tile_pool(name="psOC", bufs=3, space="PSUM") as psOC:
        for t4 in range(NT4):
            for h in range(NH):
                p_tiles = []
                for m in range(4):
                    ti = t4 * 4 + m
                    s_len = (ti + 1) * P
                    # chunk widths <=512, avoiding tails <256 (slow for fp32r)
                    widths = []
                    rem = s_len
                    while rem > 0:
                        if rem == 128 * 5:
                            w_ = 384
                        elif rem % 512 == 128 and rem > 512:
                            w_ = 384
                        else:
                            w_ = min(512, rem)
                        widths.append(w_)
                        rem -= w_
                    nch = len(widths)
                    p_ti = pp.tile([P, T], bf16, tag="p", name="p_ti")[:, :s_len]
                    rsp = sst.tile([P, 4], f32, tag="rsp", name="rsp")
                    off = 0
                    for cj, w in enumerate(widths):
                        ps_s = psS.tile([P, 512], f32, tag="s",
                                        name="ps_s")[:, :w]
                        nc.tensor.matmul(
                            ps_s, sb["qT_all"][:, h, ti * P:(ti + 1) * P],
                            sb["kT_all"][:, off:off + w],
                            start=True, stop=True)
                        if cj == nch - 1:  # causal mask on the diagonal block
                            nc.vector.tensor_add(
                                ps_s[:, w - P:w], ps_s[:, w - P:w],
                                sb["tril_s"])
                        nc.scalar.activation(
                            p_ti[:, off:off + w], ps_s, Exp,
                            accum_out=rsp[:, cj:cj + 1])
                        off += w
                    rs = rsp[:, 0:1]
                    if nch > 1:
                        rs = sst.tile([P, 1], f32, tag="rs", name="rs")
                        nc.vector.tensor_add(rs, rsp[:, 0:1], rsp[:, 1:2])
                        for cj in range(2, nch):
                            nc.vector.tensor_add(rs, rs, rsp[:, cj:cj + 1])
                    rcp = sst.tile([P, 1], f32, tag="rcp", name="rcp")
                    nc.vector.reciprocal(rcp, rs)
                    nc.vector.tensor_scalar_mul(p_ti, p_ti, rcp)
                    p_tiles.append(p_ti)

                # PV: O^T[d, t512] accumulated over 128-wide s chunks.
                # For s chunk sj, only t blocks with ti >= sj are causal-valid;
                # the valid region is the contiguous tail [m0*128, 512).
                ps_o = psOC.tile([P, 512], f32, tag="oy", name="ps_o")
                n_sj = t4 * 4 + 4
                for sj in range(n_sj):
                    m0 = max(0, sj - t4 * 4)
                    ps_t4 = psT.tile([P, 512], bf16, tag="t", name="ps_t4")
                    for m in range(m0, 4):
                        nc.tensor.matmul(
                            ps_t4[:, m * P:(m + 1) * P],
                            p_tiles[m][:, sj * P:(sj + 1) * P],
                            sb["identb_s"], is_transpose=True,
                            start=True, stop=True)
                    pt = ptp.tile([P, 512], bf16, tag="pt", name="pt")
                    nc.vector.tensor_copy(pt[:, m0 * P:512],
                                          ps_t4[:, m0 * P:512])
                    nc.tensor.matmul(ps_o[:, m0 * P:512], sb["v_all"][:, sj, :],
                                     pt[:, m0 * P:512], start=(sj == 0),
                                     stop=(sj == n_sj - 1),
                                     skip_group_check=True)
                nc.scalar.copy(sb["aoT_s"][:, h, t4 * 512:(t4 + 1) * 512], ps_o)

            # O-projection for this t4 block (all heads now available)
            for u in range(4):
                m = t4 * 4 + u
                for cc in range(4):
                    ps_y = psOC.tile([P, 512], f32, tag="oy", name="ps_y")
                    for h in range(NH):
                        nc.tensor.matmul(
                            ps_y, sb["aoT_s"][:, h, m * P:(m + 1) * P],
                            sb["wo_s"][:, h, cc * 512:(cc + 1) * 512],
                            start=(h == 0), stop=(h == NH - 1))
                    y_sb = yp.tile([P, 512], f32, tag="y_sb", name="y_sb")
                    nc.vector.tensor_copy(y_sb, ps_y)
                    nc.sync.dma_start(
                        out=io["y"][m * P:(m + 1) * P, cc * 512:(cc + 1) * 512],
                        in_=y_sb)


def _phase_c(nc, tc, io, sb):
    """Folded into _phase_b (t4-outer)."""


def build_program(phases="ABC"):
    nc = bacc.Bacc("TRN2", target_bir_lowering=False, debug=False)

    io = {
        "xT": nc.dram_tensor("xT", [C, T], f32r, kind="ExternalInput").ap(),
        "wq": nc.dram_tensor("wq", [C, NH * HD], f32r, kind="ExternalInput").ap(),
        "wkv": nc.dram_tensor("wkv", [C, 2 * HD], f32r, kind="ExternalInput").ap(),
        "wo": nc.dram_tensor("wo", [NH * HD, C], bf16, kind="ExternalInput").ap(),
        "cosx": nc.dram_tensor("cosx", [T, HD // 2], f32, kind="ExternalInput").ap(),
        "sinx": nc.dram_tensor("sinx", [T, HD // 2], f32, kind="ExternalInput").ap(),
        "tril": nc.dram_tensor("tril", [P, P], f32, kind="ExternalInput").ap(),
        "ident": nc.dram_tensor("ident", [P, P], f32r, kind="ExternalInput").ap(),
        "identb": nc.dram_tensor("identb", [P, P], bf16, kind="ExternalInput").ap(),
        "y": nc.dram_tensor("y", [T, C], f32, kind="ExternalOutput").ap(),
    }

    with tile.TileContext(nc) as tc:
        with tc.tile_pool(name="const", bufs=1) as const:
            sb = {}
            sb["wq_s"] = const.tile([P, NCC, NH * HD], f32r, name="wq_s")
            sb["wkv_s"] = const.tile([P, NCC, 2 * HD], f32r, name="wkv_s")
            sb["wo_s"] = const.tile([P, NH, C], bf16, name="wo_s")
            sb["wq_r"] = io["wq"].rearrange("(c p) n -> p c n", p=P)
            sb["wkv_r"] = io["wkv"].rearrange("(c p) n -> p c n", p=P)
            sb["wo_r"] = io["wo"].rearrange("(h p) n -> p h n", p=P)
            sb["tril_s"] = const.tile([P, P], f32, name="tril_s")
            nc.sync.dma_start(out=sb["tril_s"], in_=io["tril"])
            sb["ident_s"] = const.tile([P, P], f32r, name="ident_s")
            nc.sync.dma_start(out=sb["ident_s"], in_=io["ident"])
            sb["identb_s"] = const.tile([P, P], bf16, name="identb_s")
            nc.sync.dma_start(out=sb["identb_s"], in_=io["identb"])

            sb["qT_all"] = const.tile([P, NH, T], f32r, name="qT_all")
            sb["kT_all"] = const.tile([P, T], f32r, name="kT_all")
            sb["v_all"] = const.tile([P, NT, HD], bf16, name="v_all")
            sb["aoT_s"] = const.tile([P, NH, T], bf16, name="aoT_s")

            if "A" in phases:
                _phase_a(nc, tc, io, sb)
            if "B" in phases:
                _phase_b(nc, tc, io, sb)
            if "C" not in phases:
                # debug output so the program still writes y
                with tc.tile_pool(name="dbg", bufs=1) as dbg:
                    d = dbg.tile([P, T], f32, name="d")
                    nc.vector.tensor_copy(d, sb["kT_all"].bitcast(f32))
                    nc.sync.dma_start(out=io["y"][0:P, :], in_=d)

    nc.compile()
    return nc


_PROG = None


def _get_prog():
    global _PROG
    if _PROG is None:
        _PROG = build_program()
    return _PROG


def make_in_maps(x, cos, sin, wq, wk, wv, wo):
    """Shard full inputs into 8 per-core input dicts."""
    cosf = np.ascontiguousarray(cos.reshape(T, HD // 2)).astype(np.float32)
    sinf = np.ascontiguousarray(sin.reshape(T, HD // 2)).astype(np.float32)
    ii, jj = np.indices((P, P))
    tril = np.where(jj <= ii, 0.0, -1e30).astype(np.float32)
    ident = np.eye(P, dtype=np.float32)
    identb = np.eye(P, dtype=np.float32).astype(ml_dtypes.bfloat16)

    in_maps = []
    for g in range(8):
        b, kv = divmod(g, 4)
        sl4 = slice(kv * NH * HD, (kv + 1) * NH * HD)   # 512 wide
        sl1 = slice(kv * HD, (kv + 1) * HD)             # 128 wide
        in_maps.append({
            "xT": np.ascontiguousarray(x[b].T).astype(np.float32),
            "wq": np.ascontiguousarray(wq[:, sl4]).astype(np.float32),
            "wkv": np.ascontiguousarray(
                np.concatenate([wk[:, sl1], wv[:, sl1]], axis=1)).astype(np.float32),
            "wo": np.ascontiguousarray(wo[sl4, :]).astype(ml_dtypes.bfloat16),
            "cosx": cosf, "sinx": sinf,
            "tril": tril, "ident": ident, "identb": identb,
        })
    return in_maps


def kernel(x, cos, sin, wq, wk, wv, wo, window_size=0):
    x = np.asarray(x); cos = np.asarray(cos); sin = np.asarray(sin)
    wq = np.asarray(wq); wk = np.asarray(wk); wv = np.asarray(wv)
    wo = np.asarray(wo)
    prog = _get_prog()
    in_maps = make_in_maps(x, cos, sin, wq, wk, wv, wo)
    res = run_bass_kernel_spmd(prog, in_maps, core_ids=list(range(8)))
    outs = [r["y"] for r in res.results]
    yfull = np.empty((2, T, C), dtype=np.float32)
    for b in range(2):
        yfull[b] = outs[4 * b] + outs[4 * b + 1] + outs[4 * b + 2] + outs[4 * b + 3]
    return yfull
